# revision 15
# baseline (speedup 1.0000x reference)
"""MultiHeadLatentAttention Trainium2 Bass kernel (v4: woven phases).

Sharding (8 cores): core c = (b, hg) with b = c // 2, hg = c % 2.
Each core handles batch b and head-group hg (8 of 16 heads).

v4 structure:
  - phase 0: DMA x (bf16) and pre-transpose ALL of x^T into a resident
    SBUF tile (xtall, 8 MB) while the weights stream in.
  - woven main loop, segments s=0..4: segment s emits QKV projection
    t-tiles 4s..4s+3 (s<4) interleaved with attention units for chunk
    j=s-1 (s>=1).  The scalar-engine softmax exp therefore hides under
    projection matmuls instead of pacing its own phase.
  - PSUM budget (8 banks): shared ring "prj" [128,512]f32 x2 for q/k/v
    projection outputs AND q/k transposes; "pss" [128,1024]f32 x2 for
    scores (both heads, one exp per s-chunk); "py" [65,512] x2 for PV
    accumulators and the denominator-broadcast matmul.
  - bf16 everywhere except PSUM accumulation and norm/rope arithmetic.
  - software-pipelined attention inner loop (QK of chunk i+1 before PV
    of chunk i); masks on DVE (collectives block the Pool queue).
"""

import numpy as np

import concourse.bass as bass
import concourse.mybir as mybir
import concourse.tile as tile
from concourse import bacc
from concourse.bass import ts
from concourse.masks import make_identity

F32 = mybir.dt.float32
F32R = mybir.dt.float32r
BF16 = mybir.dt.bfloat16

N_HEAD = 16
N_EMBD = 2048
N_LATENT = 1024
HEAD_DIM = 64
ROPE_BASE = 10000.0
EPS = 1e-6
N_CORES = 8

HPC = N_HEAD // 2        # heads per core = 8
DW = HPC * HEAD_DIM      # local head width = 512
TCH = 512                # t-chunk for attention moving dim


def build_nc(T=2048, C=2048, num_devices=N_CORES):
    """Build the SPMD program (identical on all cores; data differs)."""
    nc = bacc.Bacc("TRN2", target_bir_lowering=False, debug=False,
                   num_devices=num_devices)

    NT = T // 128            # t-tiles
    NCT = C // 128           # c-tiles (contraction tiles for qkv proj)
    NJ = T // TCH            # t-chunks for attention = 4
    TPS = NT // NJ           # t-tiles per segment = 4
    CH = C // 2              # out c-half width = 1024
    NL = N_LATENT // 128     # l-tiles for out proj = 8
    CCW = 512
    NCC = CH // CCW

    x_d = nc.dram_tensor("x", [T, C], BF16, kind="ExternalInput").ap()
    wqT_d = nc.dram_tensor("wqT", [C, DW], BF16, kind="ExternalInput").ap()
    wkT_d = nc.dram_tensor("wkT", [C, DW], BF16, kind="ExternalInput").ap()
    wvT_d = nc.dram_tensor("wvT", [C, DW], BF16, kind="ExternalInput").ap()
    woT_d = nc.dram_tensor("woutT", [N_LATENT, CH], BF16, kind="ExternalInput").ap()
    cos_d = nc.dram_tensor("cosf", [T, DW], BF16, kind="ExternalInput").ap()
    sin_d = nc.dram_tensor("sinf", [T, DW], BF16, kind="ExternalInput").ap()
    mask_d = nc.dram_tensor("masks", [4, 128, TCH], BF16,
                            kind="ExternalInput").ap()
    out_d = nc.dram_tensor("out_half", [T, CH], F32, kind="ExternalOutput").ap()

    groups = [[i, i + 1] for i in range(0, num_devices, 2)]

    with tile.TileContext(nc) as tc:
        with (
            tc.tile_pool(name="const", bufs=1) as constp,
            tc.tile_pool(name="big", bufs=1) as bigp,
            tc.tile_pool(name="dram", bufs=1, space=bass.MemorySpace.DRAM) as dramp,
        ):
            ident = constp.tile([128, 128], F32, tag="ident")
            make_identity(nc, ident[:])
            identb = constp.tile([128, 128], BF16, tag="identb")
            nc.vector.tensor_copy(identb[:], ident[:])
            eps_sb = constp.tile([128, 1], F32, tag="eps")
            nc.vector.memset(eps_sb[:], EPS)
            ones8 = constp.tile([128, HPC], BF16, tag="ones8")
            nc.vector.memset(ones8[:], 1.0)
            ones_f = constp.tile([128, 64], F32, tag="ones_f")
            nc.vector.memset(ones_f[:], 1.0)
            onesr = constp.tile([128, 64], F32R, tag="onesr")
            nc.vector.tensor_copy(onesr[:], ones_f[:])
            I32 = mybir.dt.int32
            magic = constp.tile([128, 4 * 16], I32, tag="magic")
            nc.vector.memset(magic[:], 0x5F3759DF)
            mask_sb = []
            for o in range(4):
                m = constp.tile([128, TCH], BF16, tag=f"mask{o}",
                                name=f"mask{o}")
                nc.sync.dma_start(m[:], mask_d[o])
                mask_sb.append(m)

            qtd = dramp.tile([DW, T], BF16, tag="qtd")
            ktd = dramp.tile([DW, T], BF16, tag="ktd")
            vd = dramp.tile([T, DW], BF16, tag="vd")
            ytl = dramp.tile([DW, T], BF16, tag="ytl")
            ytfs = []
            for hp in range(HPC // 2):
                yf = dramp.tile([256, T], BF16, tag=f"ytf{hp}", name=f"ytf{hp}")
                ytfs.append(yf)

            # persistent SBUF: weights, x^T, k^T, v(+ones)
            # (weight DMAs are chunked and emitted interleaved with the
            #  phase-0 x loads so the first projection isn't starved)
            wsb = {}
            wload = []
            for name, wd in (("q", wqT_d), ("k", wkT_d), ("v", wvT_d)):
                w = bigp.tile([128, NCT * DW], BF16, tag=f"w{name}",
                              name=f"w{name}")
                wv_ = w[:].rearrange("p (ct d) -> p ct d", d=DW)
                wsv = wd.rearrange("(ct p) d -> p ct d", p=128)
                for cch in range(4):
                    wload.append((wv_[:, 4 * cch:4 * cch + 4, :],
                                  wsv[:, 4 * cch:4 * cch + 4, :]))
                wsb[name] = w
            xtall = bigp.tile([128, NCT * T], BF16, tag="xtall")
            xtv = xtall[:].rearrange("p (ct t) -> p ct t", t=T)
            kts = []
            for hp in range(HPC // 2):
                kt = bigp.tile([128, T], BF16, tag=f"kt{hp}", name=f"kt{hp}")
                kts.append(kt)
            v65 = []
            for si in range(NT):
                v = bigp.tile([128, HPC * 65], BF16, tag=f"v65_{si}",
                              name=f"v65_{si}")
                vv = v[:].rearrange("p (h e) -> p h e", e=65)
                nc.vector.tensor_copy(
                    vv[:, :, 64:65].rearrange("p h one -> p (h one)"),
                    ones8[:])
                v65.append(v)

            # ---------------- Phase 0: x^T precompute ----------------------
            with (
                tc.tile_pool(name="p0", bufs=2) as p0,
                tc.tile_pool(name="p0ps", bufs=2, space=bass.MemorySpace.PSUM) as p0ps,
            ):
                for tt in range(NT):
                    xa = p0.tile([128, C], BF16, tag="xa")
                    nc.sync.dma_start(xa[:], x_d[ts(tt, 128), :])
                    if tt < len(wload):
                        dst_w, src_w = wload[tt]
                        nc.sync.dma_start(dst_w, src_w)
                    for g in range(NCT // 4):
                        xps = p0ps.tile([128, 512], BF16, tag="xps")
                        for bi in range(4):
                            ct = 4 * g + bi
                            nc.tensor.transpose(
                                xps[:, ts(bi, 128)], xa[:, ts(ct, 128)],
                                identb[:]
                            )
                        nc.vector.tensor_copy(
                            xtv[:, 4 * g:4 * g + 4, ts(tt, 128)],
                            xps[:].rearrange("p (b t) -> p b t", t=128),
                        )

            # ---------------- Woven main loop ------------------------------
            with (
                tc.tile_pool(name="pw", bufs=2) as pw,
                tc.tile_pool(name="pat", bufs=2) as pat,
                tc.tile_pool(name="pprj", bufs=2, space=bass.MemorySpace.PSUM) as pprj,
                tc.tile_pool(name="pss", bufs=2, space=bass.MemorySpace.PSUM) as pssp,
                tc.tile_pool(name="p2y", bufs=2, space=bass.MemorySpace.PSUM) as p2y,
            ):
                scale = 1.0 / np.sqrt(HEAD_DIM)

                tstate = {}

                def p1_tile_a(tt, ssinfo):
                    cos_t = pw.tile([128, DW], BF16, tag="cos", bufs=4)
                    sin_t = pw.tile([128, DW], BF16, tag="sin", bufs=4)
                    nc.sync.dma_start(cos_t[:], cos_d[ts(tt, 128), :])
                    nc.sync.dma_start(sin_t[:], sin_d[ts(tt, 128), :])

                    sb = {}
                    for name in ("q", "k", "v"):
                        p = pprj.tile([128, DW], F32, tag="prj",
                                      name=f"prj_{name}_{tt}")
                        for ct in range(NCT):
                            nc.tensor.matmul(
                                p[:],
                                xtall[:, ct * T + tt * 128: ct * T + tt * 128 + 128],
                                wsb[name][:, ts(ct, DW)],
                                start=(ct == 0), stop=(ct == NCT - 1),
                            )
                        if name == "v":
                            vsb = pw.tile([128, DW], BF16, tag="vsb", bufs=2)
                            nc.scalar.activation(
                                vsb[:], p[:], mybir.ActivationFunctionType.Copy
                            )
                            nc.sync.dma_start(vd[ts(tt, 128), :], vsb[:])
                        else:
                            qs = pw.tile([128, DW], BF16, tag=f"{name}s", bufs=4)
                            nc.scalar.activation(
                                qs[:], p[:], mybir.ActivationFunctionType.Copy
                            )
                            sb[name] = qs

                    ssall, i_seg = ssinfo
                    for ni, name in enumerate(("q", "k")):
                        qs = sb[name]
                        sq = pw.tile([128, DW], F32, tag="sq", bufs=2)
                        nc.gpsimd.tensor_mul(sq[:], qs[:], qs[:])
                        nc.vector.tensor_reduce(
                            ssall[:, i_seg * 16 + ni * 8:
                                  i_seg * 16 + ni * 8 + 8],
                            sq[:].rearrange("p (h d) -> p h d", d=HEAD_DIM),
                            axis=mybir.AxisListType.X,
                            op=mybir.AluOpType.add,
                        )
                    tstate[tt] = (cos_t, sin_t, sb)

                def newton_rsqrt(ssall):
                    """rfall = 1/sqrt(ssall/64 + eps), DVE only (no act table).
                    Quake-style int seed + 2 Newton steps (ms in ~[0.1,3])."""
                    W = 4 * 16
                    m = pw.tile([128, W], F32, tag="msall", bufs=2)
                    nc.vector.tensor_scalar(
                        m[:], ssall[:], 1.0 / HEAD_DIM, EPS,
                        op0=mybir.AluOpType.mult, op1=mybir.AluOpType.add)
                    hbits = pw.tile([128, W], mybir.dt.int32, tag="hbits",
                                    bufs=2)
                    nc.vector.tensor_scalar(
                        hbits[:], m[:].bitcast(mybir.dt.int32), 1, None,
                        op0=mybir.AluOpType.logical_shift_right)
                    y = pw.tile([128, W], F32, tag="yall", bufs=2)
                    nc.vector.tensor_tensor(
                        y[:].bitcast(mybir.dt.int32), magic[:], hbits[:],
                        op=mybir.AluOpType.subtract)
                    for _ in range(2):
                        a = pw.tile([128, W], F32, tag="nt_a", bufs=2)
                        nc.vector.tensor_mul(a[:], y[:], y[:])
                        nc.vector.tensor_mul(a[:], a[:], m[:])
                        nc.vector.tensor_scalar(
                            a[:], a[:], -0.5, 1.5,
                            op0=mybir.AluOpType.mult, op1=mybir.AluOpType.add)
                        nc.vector.tensor_mul(y[:], y[:], a[:])
                    return y

                def p1_tile_b(tt, rfall, i_seg):
                    cos_t, sin_t, sb = tstate.pop(tt)
                    for ni, (name, dst) in enumerate((("q", qtd), ("k", ktd))):
                        qs = sb[name]
                        rfac = rfall[:, i_seg * 16 + ni * 8:
                                     i_seg * 16 + ni * 8 + 8]
                        m2 = pw.tile([128, DW], F32, tag="m2", bufs=2)
                        qv = qs[:].rearrange("p (h two d) -> p h two d", two=2,
                                             d=HEAD_DIM // 2)
                        m2v = m2[:].rearrange("p (h two d) -> p h two d", two=2,
                                              d=HEAD_DIM // 2)
                        sv = sin_t[:].rearrange("p (h two d) -> p h two d",
                                                two=2, d=HEAD_DIM // 2)
                        nc.gpsimd.tensor_mul(m2v[:, :, 0, :], qv[:, :, 1, :],
                                             sv[:, :, 0, :])
                        nc.gpsimd.tensor_mul(m2v[:, :, 1, :], qv[:, :, 0, :],
                                             sv[:, :, 1, :])
                        m1 = pw.tile([128, DW], F32, tag="m1", bufs=2)
                        nc.gpsimd.tensor_mul(m1[:], qs[:], cos_t[:])
                        nc.gpsimd.tensor_add(m1[:], m1[:], m2[:])
                        qrb = pw.tile([128, DW], F32, tag="qrb", bufs=2)
                        for h in range(HPC):
                            nc.vector.tensor_scalar_mul(
                                qrb[:, ts(h, HEAD_DIM)],
                                m1[:, ts(h, HEAD_DIM)],
                                rfac[:, h: h + 1],
                            )
                        tp = pprj.tile([128, DW], F32, tag="prj",
                                       name=f"tp_{name}_{tt}")
                        for db in range(DW // 128):
                            nc.tensor.transpose(
                                tp[:, ts(db, 128)], qrb[:, ts(db, 128)],
                                ident[:]
                            )
                        qt = pw.tile([128, DW], BF16, tag="qt", bufs=3)
                        nc.vector.tensor_copy(qt[:], tp[:])
                        nc.sync.dma_start(
                            dst[:, ts(tt, 128)].rearrange(
                                "(db p) t -> p db t", p=128),
                            qt[:].rearrange("p (db t) -> p db t", t=128),
                        )

                def attn_unit(hp, j, q2):
                    smax = (j + 1) * (TCH // 128)
                    pys = []
                    for e in range(2):
                        pys.append(p2y.tile([65, TCH], F32, tag="py",
                                            name=f"py{e}_{hp}_{j}"))

                    def score(si):
                        pss = pssp.tile([128, 2 * TCH], F32, tag="pss")
                        for e in range(2):
                            nc.tensor.matmul(
                                pss[:, ts(e, TCH)],
                                kts[hp][64 * e: 64 * e + 64, ts(si, 128)],
                                q2[64 * e: 64 * e + 64, :],
                            )
                        pt = pat.tile([128, 2 * TCH], BF16, tag="pt", bufs=2)
                        nc.scalar.activation(
                            pt[:], pss[:],
                            mybir.ActivationFunctionType.Exp,
                            scale=scale,
                        )
                        o = si - (smax - TCH // 128)
                        if o >= 0:
                            for e in range(2):
                                nc.vector.tensor_mul(
                                    pt[:, ts(e, TCH)], pt[:, ts(e, TCH)],
                                    mask_sb[o][:])
                        return pt

                    pts = {0: score(0)}
                    for si in range(smax):
                        if si + 1 < smax:
                            pts[si + 1] = score(si + 1)
                        pt = pts.pop(si)
                        for e in range(2):
                            h = 2 * hp + e
                            nc.tensor.matmul(
                                pys[e][:],
                                v65[si][:, 65 * h: 65 * h + 65],
                                pt[:, ts(e, TCH)],
                                start=(si == 0),
                                stop=(si == smax - 1),
                            )
                    ynt = pat.tile([128, TCH], BF16, tag="ynt", bufs=2)
                    for e in range(2):
                        ystage = pat.tile([65, TCH], F32R, tag="ystage", bufs=2)
                        nc.vector.tensor_copy(ystage[:], pys[e][:])
                        bc = p2y.tile([65, TCH], F32, tag="py",
                                      name=f"bc{e}_{hp}_{j}")
                        nc.tensor.matmul(
                            bc[0:64, :], onesr[64:65, :], ystage[64:65, :]
                        )
                        bcr = pat.tile([64, TCH], F32, tag="bcr", bufs=1)
                        nc.vector.reciprocal(bcr[:], bc[0:64, :])
                        nc.gpsimd.tensor_mul(
                            ynt[64 * e: 64 * e + 64, :],
                            ystage[0:64, :], bcr[:]
                        )
                    nc.sync.dma_start(ytl[ts(hp, 128), ts(j, TCH)], ynt[:])

                # segments
                for seg in range(NJ + 1):
                    tiles = list(range(TPS * seg, TPS * seg + TPS)) if seg < NJ else []
                    j = seg - 1
                    q2s = {}
                    if seg >= 1:
                        # k/v chunk loads + q prefetch for chunk j (written
                        # by the previous segment's tiles)
                        for hp in range(HPC // 2):
                            nc.sync.dma_start(
                                kts[hp][:, ts(j, TCH)],
                                ktd[ts(hp, 128), ts(j, TCH)])
                        for si in range(TPS * j, TPS * j + TPS):
                            vv = v65[si][:].rearrange("p (h e) -> p h e", e=65)
                            nc.sync.dma_start(
                                vv[:, :, 0:64],
                                vd[ts(si, 128), :]
                                .rearrange("p (h d) -> p h d", d=HEAD_DIM))
                        for hp in range(HPC // 2):
                            q2 = pat.tile([128, TCH], BF16, tag="q2", bufs=4)
                            nc.sync.dma_start(
                                q2[:], qtd[ts(hp, 128), ts(j, TCH)])
                            q2s[hp] = q2
                    ssall = None
                    if tiles:
                        ssall = pw.tile([128, 4 * 16], F32, tag="ssall",
                                        bufs=2, name=f"ssall{seg}")
                    # interleave: tile-part-a, attn, tile-part-a, attn, ...
                    for i in range(max(len(tiles), 4 if seg >= 1 else 0)):
                        if i < len(tiles):
                            p1_tile_a(tiles[i], (ssall, i))
                        if seg >= 1 and i < 4:
                            attn_unit(i, j, q2s[i])
                            if seg == NJ:
                                nc.gpsimd.collective_compute(
                                    "AllGather",
                                    mybir.AluOpType.bypass,
                                    replica_groups=groups,
                                    ins=[ytl[ts(i, 128), :]],
                                    outs=[ytfs[i][:]],
                                )
                    # batched DVE rsqrt (no act-table switches) + rope/store
                    if tiles:
                        rfall = newton_rsqrt(ssall)
                        for i, t_ in enumerate(tiles):
                            p1_tile_b(t_, rfall, i)

            # ---------------- Phase 3: out projection ---------------------
            with (
                tc.tile_pool(name="p3w", bufs=1) as p3w,
                tc.tile_pool(name="p3", bufs=3) as p3,
                tc.tile_pool(name="p3y", bufs=1) as p3y,
                tc.tile_pool(name="p3ps", bufs=3, space=bass.MemorySpace.PSUM) as p3ps,
            ):
                wo = p3w.tile([128, NL * CH], BF16, tag="wo")
                nc.sync.dma_start(
                    wo[:].rearrange("p (lt c) -> p lt c", c=CH),
                    woT_d.rearrange("(lt p) c -> p lt c", p=128),
                )
                yts = []
                for lt in range(NL):
                    y = p3y.tile([128, T], BF16, tag=f"yr{lt}", name=f"yr{lt}")
                    nc.sync.dma_start(
                        y[:], ytfs[lt % 4][(lt // 4) * 128:(lt // 4 + 1) * 128, :])
                    yts.append(y)
                for tt in range(NT):
                    for cc in range(NCC):
                        po = p3ps.tile([128, CCW], F32, tag="po")
                        for i, lt in enumerate(range(NL)):
                            nc.tensor.matmul(
                                po[:],
                                yts[lt][:, ts(tt, 128)],
                                wo[:, lt * CH + cc * CCW: lt * CH + (cc + 1) * CCW],
                                start=(i == 0),
                                stop=(i == NL - 1),
                            )
                        osb = p3.tile([128, CCW], F32, tag="osb")
                        nc.scalar.activation(
                            osb[:], po[:], mybir.ActivationFunctionType.Copy
                        )
                        nc.sync.dma_start(
                            out_d[ts(tt, 128), ts(cc, CCW)], osb[:]
                        )

    nc.compile()
    return nc


def host_tables(T=2048):
    inv_freq = 1.0 / (ROPE_BASE ** (np.arange(0, HEAD_DIM, 2, dtype=np.float32)
                                    / HEAD_DIM))
    t = np.arange(T, dtype=np.float32)
    freqs = np.outer(t, inv_freq)
    cos = np.cos(freqs).astype(np.float32)
    sin = np.sin(freqs).astype(np.float32)
    cosf = np.tile(np.concatenate([cos, cos], axis=1), (1, HPC))
    sinf = np.tile(np.concatenate([sin, -sin], axis=1), (1, HPC))
    masks = np.zeros((4, 128, TCH), dtype=np.float32)
    for i, o in enumerate(range(0, TCH, 128)):
        masks[i] = (np.arange(TCH)[None, :] >=
                    (np.arange(128)[:, None] + o)).astype(np.float32)
    return np.ascontiguousarray(cosf), np.ascontiguousarray(sinf), masks


def make_in_maps(x, w_qkv, w_out, T=2048, num_devices=N_CORES):
    from ml_dtypes import bfloat16

    x = np.asarray(x, dtype=np.float32)
    w_qkv = np.asarray(w_qkv, dtype=np.float32)
    w_out = np.asarray(w_out, dtype=np.float32)
    C = x.shape[-1]
    cosf, sinf, masks = host_tables(T)
    masks_b = masks.astype(bfloat16)
    in_maps = []
    for c in range(num_devices):
        b, hg = c // 2, c % 2
        sl = slice(hg * DW, (hg + 1) * DW)
        in_maps.append({
            "x": np.ascontiguousarray(x[b].astype(bfloat16)),
            "wqT": np.ascontiguousarray(
                w_qkv[0 * N_LATENT:, :][sl].T.astype(bfloat16)),
            "wkT": np.ascontiguousarray(
                w_qkv[1 * N_LATENT:, :][sl].T.astype(bfloat16)),
            "wvT": np.ascontiguousarray(
                w_qkv[2 * N_LATENT:, :][sl].T.astype(bfloat16)),
            "woutT": np.ascontiguousarray(
                w_out[hg * C // 2:(hg + 1) * C // 2, :].T.astype(bfloat16)),
            "cosf": cosf.astype(bfloat16),
            "sinf": sinf.astype(bfloat16),
            "masks": masks_b,
        })
    return in_maps


_NC = None


def kernel(x, w_qkv, w_out):
    global _NC
    if _NC is None:
        _NC = build_nc()
    from concourse.bass_utils import run_bass_kernel_spmd
    in_maps = make_in_maps(x, w_qkv, w_out)
    res = run_bass_kernel_spmd(_NC, in_maps, list(range(N_CORES))).results
    B, T = 4, 2048
    out = np.empty((B, T, N_EMBD), dtype=np.float32)
    for c in range(N_CORES):
        b, hg = c // 2, c % 2
        out[b, :, hg * N_EMBD // 2:(hg + 1) * N_EMBD // 2] = res[c]["out_half"]
    return out


# revision 16
# speedup vs baseline: 1.0549x; 1.0549x over previous
"""MultiHeadLatentAttention Trainium2 Bass kernel (optimized).

Sharding (8 cores): core c = (b, hg) with b = c // 2, hg = c % 2.
Each core handles batch b and head-group hg (8 of 16 heads):
  - QKV projection for its heads (weights pre-sliced+transposed+bf16 on host)
  - qk rmsnorm + RoPE + causal attention for its 8 heads
  - pairwise AllGather of y^T (bf16) between (2b, 2b+1)
  - out-projection for c-half hg*1024:(hg+1)*1024 with the full 16 heads
Key optimizations vs v1:
  - bf16 for all matmul operands (x^T, w, q^T, k^T, v, probs, y, w_out);
    PSUM accumulation stays fp32.
  - attention: one [128,1024] PSUM score tile per s-chunk covering both
    heads of the pair; ONE exp activation + ONE mask multiply per chunk.
  - software-pipelined attention inner loop (QK of chunk i+1 issued before
    PV of chunk i) so the scalar-engine exp hides under PE work.
  - rmsnorm stats on Pool(sq)+DVE(reduce), rope on Pool, scale-apply on DVE
    with bf16 output; fewer/larger DMAs.
"""

import numpy as np

import concourse.bass as bass
import concourse.mybir as mybir
import concourse.tile as tile
from concourse import bacc
from concourse.bass import ts
from concourse.masks import make_identity

F32 = mybir.dt.float32
F32R = mybir.dt.float32r
BF16 = mybir.dt.bfloat16

N_HEAD = 16
N_EMBD = 2048
N_LATENT = 1024
HEAD_DIM = 64
ROPE_BASE = 10000.0
EPS = 1e-6
N_CORES = 8

HPC = N_HEAD // 2        # heads per core = 8
DW = HPC * HEAD_DIM      # local head width = 512
TCH = 512                # t-chunk for attention moving dim


def build_nc(T=2048, C=2048, num_devices=N_CORES):
    """Build the SPMD program (identical on all cores; data differs)."""
    nc = bacc.Bacc("TRN2", target_bir_lowering=False, debug=False,
                   num_devices=num_devices)

    NT = T // 128            # t-tiles
    NCT = C // 128           # c-tiles (contraction tiles for qkv proj)
    NJ = T // TCH            # t-chunks for attention
    CH = C // 2              # out c-half width = 1024
    NL = N_LATENT // 128     # l-tiles for out proj = 8
    CCW = 512                # out column chunk
    NCC = CH // CCW

    x_d = nc.dram_tensor("x", [T, C], BF16, kind="ExternalInput").ap()
    wqT_d = nc.dram_tensor("wqT", [C, DW], BF16, kind="ExternalInput").ap()
    wkT_d = nc.dram_tensor("wkT", [C, DW], BF16, kind="ExternalInput").ap()
    wvT_d = nc.dram_tensor("wvT", [C, DW], BF16, kind="ExternalInput").ap()
    woT_d = nc.dram_tensor("woutT", [N_LATENT, CH], BF16, kind="ExternalInput").ap()
    cos_d = nc.dram_tensor("cosf", [T, DW], F32, kind="ExternalInput").ap()
    sin_d = nc.dram_tensor("sinf", [T, DW], F32, kind="ExternalInput").ap()
    mask_d = nc.dram_tensor("masks", [4, 128, 2 * TCH], BF16,
                            kind="ExternalInput").ap()
    out_d = nc.dram_tensor("out_half", [T, CH], F32, kind="ExternalOutput").ap()

    groups = [[i, i + 1] for i in range(0, num_devices, 2)]

    with tile.TileContext(nc) as tc:
        with (
            tc.tile_pool(name="const", bufs=1) as constp,
            tc.tile_pool(name="dram", bufs=1, space=bass.MemorySpace.DRAM) as dramp,
        ):
            ident = constp.tile([128, 128], F32, tag="ident")
            make_identity(nc, ident[:])
            identr = constp.tile([128, 128], F32R, tag="identr")
            nc.vector.tensor_copy(identr[:], ident[:])
            identb = constp.tile([128, 128], BF16, tag="identb")
            nc.vector.tensor_copy(identb[:], ident[:])
            eps_sb = constp.tile([128, 1], F32, tag="eps")
            nc.vector.memset(eps_sb[:], EPS)
            ones8 = constp.tile([128, HPC], BF16, tag="ones8")
            nc.vector.memset(ones8[:], 1.0)
            ones_f = constp.tile([128, 64], F32, tag="ones_f")
            nc.vector.memset(ones_f[:], 1.0)
            onesr = constp.tile([128, 64], F32R, tag="onesr")
            nc.vector.tensor_copy(onesr[:], ones_f[:])
            mask_sb = []
            for o in range(4):
                m = constp.tile([128, 2 * TCH], BF16, tag=f"mask{o}",
                                name=f"mask{o}")
                nc.sync.dma_start(m[:], mask_d[o])
                mask_sb.append(m)

            qtd = dramp.tile([DW, T], BF16, tag="qtd")
            ktd = dramp.tile([DW, T], BF16, tag="ktd")
            vd = dramp.tile([T, DW], BF16, tag="vd")
            ytl = dramp.tile([DW, T], BF16, tag="ytl")
            ytfs = []
            for hp in range(HPC // 2):
                yf = dramp.tile([256, T], BF16, tag=f"ytf{hp}", name=f"ytf{hp}")
                ytfs.append(yf)

            # ---------------- Phase 1: QKV + rmsnorm + rope + transpose ----
            with (
                tc.tile_pool(name="p1w", bufs=1) as p1w,
                tc.tile_pool(name="p1", bufs=2) as p1,
                tc.tile_pool(name="p1ps", bufs=2, space=bass.MemorySpace.PSUM) as p1ps,
                tc.tile_pool(name="p1qk", bufs=1, space=bass.MemorySpace.PSUM) as p1qk,
                tc.tile_pool(name="p1v", bufs=1, space=bass.MemorySpace.PSUM) as p1v,
                tc.tile_pool(name="p1tp", bufs=2, space=bass.MemorySpace.PSUM) as p1tp,
            ):
                wsb = {}
                for name, wd in (("q", wqT_d), ("k", wkT_d), ("v", wvT_d)):
                    w = p1w.tile([128, NCT * DW], BF16, tag=f"w{name}",
                                 name=f"w{name}")
                    nc.sync.dma_start(
                        w[:].rearrange("p (ct d) -> p ct d", d=DW),
                        wd.rearrange("(ct p) d -> p ct d", p=128),
                    )
                    wsb[name] = w

                for tt in range(NT):
                    xa = p1.tile([128, C], BF16, tag="xa")
                    nc.sync.dma_start(xa[:], x_d[ts(tt, 128), :])
                    cos_t = p1.tile([128, DW], F32, tag="cos")
                    sin_t = p1.tile([128, DW], F32, tag="sin")
                    nc.sync.dma_start(cos_t[:], cos_d[ts(tt, 128), :])
                    nc.sync.dma_start(sin_t[:], sin_d[ts(tt, 128), :])

                    # x^T for this t-tile: [c, 128t] as NCT column blocks, bf16
                    xt = p1.tile([128, NCT * 128], BF16, tag="xt")
                    for g in range((NCT + 3) // 4):
                        xps = p1ps.tile([128, 512], BF16, tag="xps")
                        for bi in range(4):
                            ct = 4 * g + bi
                            nc.tensor.transpose(
                                xps[:, ts(bi, 128)], xa[:, ts(ct, 128)], identb[:]
                            )
                        nc.vector.tensor_copy(
                            xt[:, ts(g, 512)], xps[:]
                        )

                    # qk into one [128,1024] psum (q cols 0:512, k 512:1024)
                    pqk = p1qk.tile([128, 1024], F32, tag="pqk")
                    pv = p1v.tile([128, DW], F32, tag="pv")
                    for ct in range(NCT):
                        nc.tensor.matmul(
                            pqk[:, 0:DW], xt[:, ts(ct, 128)],
                            wsb["q"][:, ts(ct, DW)],
                            start=(ct == 0), stop=(ct == NCT - 1),
                        )
                    for ct in range(NCT):
                        nc.tensor.matmul(
                            pqk[:, DW:2 * DW], xt[:, ts(ct, 128)],
                            wsb["k"][:, ts(ct, DW)],
                            start=(ct == 0), stop=(ct == NCT - 1),
                        )
                    for ct in range(NCT):
                        nc.tensor.matmul(
                            pv[:], xt[:, ts(ct, 128)],
                            wsb["v"][:, ts(ct, DW)],
                            start=(ct == 0), stop=(ct == NCT - 1),
                        )

                    # V: evacuate (cast bf16) to DRAM
                    vsb = p1.tile([128, DW], BF16, tag="vsb", bufs=4)
                    nc.scalar.activation(
                        vsb[:], pv[:], mybir.ActivationFunctionType.Copy
                    )
                    nc.sync.dma_start(vd[ts(tt, 128), :], vsb[:])

                    # rmsnorm + rope for q, k
                    for qi, dst in ((0, qtd), (1, ktd)):
                        # evacuate to SBUF fp32
                        qs = p1.tile([128, DW], F32, tag="qs", bufs=3)
                        nc.scalar.activation(
                            qs[:], pqk[:, qi * DW:(qi + 1) * DW],
                            mybir.ActivationFunctionType.Copy,
                        )
                        # sumsq per head: Pool square, DVE segment-reduce
                        sq = p1.tile([128, DW], F32, tag="sq", bufs=3)
                        nc.gpsimd.tensor_mul(sq[:], qs[:], qs[:])
                        ss = p1.tile([128, HPC], F32, tag="ss", bufs=3)
                        nc.vector.tensor_reduce(
                            ss[:],
                            sq[:].rearrange("p (h d) -> p h d", d=HEAD_DIM),
                            axis=mybir.AxisListType.X,
                            op=mybir.AluOpType.add,
                        )
                        sig = p1.tile([128, HPC], F32, tag="sig", bufs=3)
                        nc.scalar.activation(
                            sig[:], ss[:],
                            mybir.ActivationFunctionType.Sqrt,
                            bias=eps_sb[:], scale=1.0 / HEAD_DIM,
                        )
                        rfac = p1.tile([128, HPC], F32, tag="rfac", bufs=3)
                        nc.vector.reciprocal(rfac[:], sig[:])
                        # rope on Pool: m2 = swap(qs)*sinf, m1 = qs*cos, m3 sum
                        m2 = p1.tile([128, DW], F32, tag="m2", bufs=3)
                        qv = qs[:].rearrange("p (h two d) -> p h two d", two=2,
                                             d=HEAD_DIM // 2)
                        m2v = m2[:].rearrange("p (h two d) -> p h two d", two=2,
                                              d=HEAD_DIM // 2)
                        sv = sin_t[:].rearrange("p (h two d) -> p h two d",
                                                two=2, d=HEAD_DIM // 2)
                        nc.gpsimd.tensor_mul(m2v[:, :, 0, :], qv[:, :, 1, :],
                                             sv[:, :, 0, :])
                        nc.gpsimd.tensor_mul(m2v[:, :, 1, :], qv[:, :, 0, :],
                                             sv[:, :, 1, :])
                        m1 = p1.tile([128, DW], F32, tag="m1", bufs=3)
                        nc.gpsimd.tensor_mul(m1[:], qs[:], cos_t[:])
                        m3 = p1.tile([128, DW], F32, tag="m3", bufs=3)
                        nc.gpsimd.tensor_add(m3[:], m1[:], m2[:])
                        # apply rmsnorm scale per head; cast to bf16
                        qrb = p1.tile([128, DW], BF16, tag="qrb", bufs=3)
                        for h in range(HPC):
                            nc.vector.tensor_scalar_mul(
                                qrb[:, ts(h, HEAD_DIM)],
                                m3[:, ts(h, HEAD_DIM)],
                                rfac[:, h: h + 1],
                            )
                        # transpose to [d, t] (bf16) and store with one DMA
                        tps = p1tp.tile([128, 512], BF16, tag="tps")
                        for db in range(DW // 128):
                            nc.tensor.transpose(
                                tps[:, ts(db, 128)], qrb[:, ts(db, 128)],
                                identb[:]
                            )
                        qt = p1.tile([128, DW], BF16, tag="qt", bufs=4)
                        nc.vector.tensor_copy(qt[:], tps[:])
                        nc.sync.dma_start(
                            dst[:, ts(tt, 128)].rearrange(
                                "(db p) t -> p db t", p=128),
                            qt[:].rearrange("p (db t) -> p db t", t=128),
                        )

            # ---------------- Phase 2: attention --------------------------
            with (
                tc.tile_pool(name="p2kv", bufs=1) as p2kv,
                tc.tile_pool(name="p2", bufs=4) as p2,
                tc.tile_pool(name="p2s", bufs=2, space=bass.MemorySpace.PSUM) as p2s,
                tc.tile_pool(name="p2y", bufs=1, space=bass.MemorySpace.PSUM) as p2y,
                tc.tile_pool(name="p2bc", bufs=2, space=bass.MemorySpace.PSUM) as p2bc,
            ):
                kts = []
                for hp in range(HPC // 2):
                    kt = p2kv.tile([128, T], BF16, tag=f"kt{hp}", name=f"kt{hp}")
                    nc.sync.dma_start(kt[:], ktd[ts(hp, 128), :])
                    kts.append(kt)
                v65 = []
                for si in range(NT):
                    v = p2kv.tile([128, HPC * 65], BF16, tag=f"v65_{si}",
                                  name=f"v65_{si}")
                    vv = v[:].rearrange("p (h e) -> p h e", e=65)
                    nc.vector.tensor_copy(
                        vv[:, :, 64:65].rearrange("p h one -> p (h one)"),
                        ones8[:])
                    nc.sync.dma_start(vv[:, :, 0:64], vd[ts(si, 128), :]
                                      .rearrange("p (h d) -> p h d", d=HEAD_DIM))
                    v65.append(v)

                scale = 1.0 / np.sqrt(HEAD_DIM)

                for hp in range(HPC // 2):
                    for j in range(NJ):
                        q2 = p2.tile([128, TCH], BF16, tag="q2", bufs=2)
                        nc.sync.dma_start(q2[:], qtd[ts(hp, 128), ts(j, TCH)])
                        smax = (j + 1) * (TCH // 128)
                        pys = []
                        for e in range(2):
                            pys.append(p2y.tile([65, TCH], F32, tag=f"py{e}",
                                                name=f"py{e}"))

                        def score(si):
                            pss = p2s.tile([128, 2 * TCH], F32, tag="pss")
                            for e in range(2):
                                nc.tensor.matmul(
                                    pss[:, ts(e, TCH)],
                                    kts[hp][64 * e: 64 * e + 64, ts(si, 128)],
                                    q2[64 * e: 64 * e + 64, :],
                                )
                            pt = p2.tile([128, 2 * TCH], BF16, tag="pt", bufs=3)
                            nc.scalar.activation(
                                pt[:], pss[:],
                                mybir.ActivationFunctionType.Exp,
                                scale=scale,
                            )
                            o = si - (smax - TCH // 128)
                            if o >= 0:
                                nc.vector.tensor_mul(pt[:], pt[:], mask_sb[o][:])
                            return pt

                        pts = {0: score(0)}
                        for si in range(smax):
                            if si + 1 < smax:
                                pts[si + 1] = score(si + 1)
                            pt = pts.pop(si)
                            for e in range(2):
                                h = 2 * hp + e
                                nc.tensor.matmul(
                                    pys[e][:],
                                    v65[si][:, 65 * h: 65 * h + 65],
                                    pt[:, ts(e, TCH)],
                                    start=(si == 0),
                                    stop=(si == smax - 1),
                                )
                        ynt = p2.tile([128, TCH], BF16, tag="ynt", bufs=2)
                        for e in range(2):
                            ystage = p2.tile([65, TCH], F32R, tag="ystage",
                                             bufs=2)
                            nc.vector.tensor_copy(ystage[:], pys[e][:])
                            bc = p2bc.tile([64, TCH], F32, tag="bc")
                            nc.tensor.matmul(
                                bc[:], onesr[64:65, :], ystage[64:65, :]
                            )
                            bcr = p2.tile([64, TCH], F32, tag="bcr", bufs=2)
                            nc.vector.reciprocal(bcr[:], bc[:])
                            nc.vector.tensor_mul(
                                ynt[64 * e: 64 * e + 64, :],
                                ystage[0:64, :], bcr[:]
                            )
                        nc.sync.dma_start(
                            ytl[ts(hp, 128), ts(j, TCH)], ynt[:]
                        )
                    nc.gpsimd.collective_compute(
                        "AllGather",
                        mybir.AluOpType.bypass,
                        replica_groups=groups,
                        ins=[ytl[ts(hp, 128), :]],
                        outs=[ytfs[hp][:]],
                    )

            # ---------------- Phase 3: out projection ---------------------
            with (
                tc.tile_pool(name="p3w", bufs=1) as p3w,
                tc.tile_pool(name="p3", bufs=3) as p3,
                tc.tile_pool(name="p3y", bufs=1) as p3y,
                tc.tile_pool(name="p3ps", bufs=3, space=bass.MemorySpace.PSUM) as p3ps,
            ):
                wo = p3w.tile([128, NL * CH], BF16, tag="wo")
                nc.sync.dma_start(
                    wo[:].rearrange("p (lt c) -> p lt c", c=CH),
                    woT_d.rearrange("(lt p) c -> p lt c", p=128),
                )
                yts = []
                for lt in range(NL):
                    y = p3y.tile([128, T], BF16, tag=f"yr{lt}", name=f"yr{lt}")
                    nc.sync.dma_start(
                        y[:], ytfs[lt % 4][(lt // 4) * 128:(lt // 4 + 1) * 128, :])
                    yts.append(y)
                for tt in range(NT):
                    for cc in range(NCC):
                        po = p3ps.tile([128, CCW], F32, tag="po")
                        for i, lt in enumerate(range(NL)):
                            nc.tensor.matmul(
                                po[:],
                                yts[lt][:, ts(tt, 128)],
                                wo[:, lt * CH + cc * CCW: lt * CH + (cc + 1) * CCW],
                                start=(i == 0),
                                stop=(i == NL - 1),
                            )
                        osb = p3.tile([128, CCW], F32, tag="osb")
                        nc.scalar.activation(
                            osb[:], po[:], mybir.ActivationFunctionType.Copy
                        )
                        nc.sync.dma_start(
                            out_d[ts(tt, 128), ts(cc, CCW)], osb[:]
                        )

    nc.compile()
    return nc


def host_tables(T=2048):
    inv_freq = 1.0 / (ROPE_BASE ** (np.arange(0, HEAD_DIM, 2, dtype=np.float32)
                                    / HEAD_DIM))
    t = np.arange(T, dtype=np.float32)
    freqs = np.outer(t, inv_freq)
    cos = np.cos(freqs).astype(np.float32)
    sin = np.sin(freqs).astype(np.float32)
    cosf = np.tile(np.concatenate([cos, cos], axis=1), (1, HPC))
    sinf = np.tile(np.concatenate([sin, -sin], axis=1), (1, HPC))
    masks = np.zeros((4, 128, TCH), dtype=np.float32)
    for i, o in enumerate(range(0, TCH, 128)):
        masks[i] = (np.arange(TCH)[None, :] >=
                    (np.arange(128)[:, None] + o)).astype(np.float32)
    masks2 = np.concatenate([masks, masks], axis=2)  # same mask for 2 heads
    return np.ascontiguousarray(cosf), np.ascontiguousarray(sinf), masks2


def make_in_maps(x, w_qkv, w_out, T=2048, num_devices=N_CORES):
    from ml_dtypes import bfloat16

    x = np.asarray(x, dtype=np.float32)
    w_qkv = np.asarray(w_qkv, dtype=np.float32)
    w_out = np.asarray(w_out, dtype=np.float32)
    C = x.shape[-1]
    cosf, sinf, masks = host_tables(T)
    masks_b = masks.astype(bfloat16)
    in_maps = []
    for c in range(num_devices):
        b, hg = c // 2, c % 2
        sl = slice(hg * DW, (hg + 1) * DW)
        in_maps.append({
            "x": np.ascontiguousarray(x[b].astype(bfloat16)),
            "wqT": np.ascontiguousarray(
                w_qkv[0 * N_LATENT:, :][sl].T.astype(bfloat16)),
            "wkT": np.ascontiguousarray(
                w_qkv[1 * N_LATENT:, :][sl].T.astype(bfloat16)),
            "wvT": np.ascontiguousarray(
                w_qkv[2 * N_LATENT:, :][sl].T.astype(bfloat16)),
            "woutT": np.ascontiguousarray(
                w_out[hg * C // 2:(hg + 1) * C // 2, :].T.astype(bfloat16)),
            "cosf": cosf,
            "sinf": sinf,
            "masks": masks_b,
        })
    return in_maps


_NC = None


def kernel(x, w_qkv, w_out):
    global _NC
    if _NC is None:
        _NC = build_nc()
    from concourse.bass_utils import run_bass_kernel_spmd
    in_maps = make_in_maps(x, w_qkv, w_out)
    res = run_bass_kernel_spmd(_NC, in_maps, list(range(N_CORES))).results
    B, T = 4, 2048
    out = np.empty((B, T, N_EMBD), dtype=np.float32)
    for c in range(N_CORES):
        b, hg = c // 2, c % 2
        out[b, :, hg * N_EMBD // 2:(hg + 1) * N_EMBD // 2] = res[c]["out_half"]
    return out
